# revision 51
# baseline (speedup 1.0000x reference)
"""Multi-head attention (causal, per-head projections) on 8 trn2 NeuronCores.

Sharding: core c = (batch b = c//2, head-quad = c%2). Each core computes its 4
heads over all 2048 queries of its batch (identical static causal structure on
every core -> one SPMD program). No on-device collective: each core emits the
PARTIAL output projection Sum_{its 4 heads} ctxn_h @ (Wh_h@Wo_h) for the full
window; the host sums the two partials of each batch pair and adds the fused
bias. This keeps every PE instruction dependent only on core-local data.

All compute in bf16 matmuls (f32 PSUM accumulate):
  X^T tiles -> qT/kT = W.T @ X^T, v natural = (X^T chunks).T @ Wv
  scoresT[k, q] = kT.T @ qT      (k on partitions -> softmax sum via matmul;
                                  diagonal tiles sliced to the causal columns)
  attnT = exp(scoresT/8)         (ACT, bf16 out); diag 128-block gets a
                                  strided in-place DVE multiply by the
                                  lower-tri 0/1 mask (both heads, one op)
  ctxT_aug = [v*keep | keep].T @ attnT  (row 64 = softmax denominators)
  ctxn_h = ctxT_h * bcast(1/rowsum_h)   (PE broadcast of DVE reciprocal)
  part_out = sum_{local h} ctxn_h.T @ (Wh_h @ Wo_h)   (fused host-side)

Scores run a few tiles ahead of ctx (software pipeline), the normalize chain
of a head-pair is deferred into the next pair's score stream, and projections
of window w+1 plus the partial-output chunks of window w-1 are injected
between attention tiles of window w, keeping the PE stream dense (p-state).
DMAs ride the SP/Pool queues only so the ACT engine runs nothing but exp.
"""

import os

import numpy as np

import concourse.bass as bass
import concourse.tile as tile
from concourse import bacc, mybir
from concourse import bass_utils

B, S, D, H, DK, DV = 4, 2048, 512, 8, 64, 64
HL = H // 2          # heads per core (4)
NW = S // 512        # 512-wide q windows (4)
NT = S // 128        # 128-row k tiles (16)
F32 = mybir.dt.float32
BF16 = mybir.dt.bfloat16
EXP = mybir.ActivationFunctionType.Exp


def build_program():
    nc = bacc.Bacc("TRN2", target_bir_lowering=False, debug=False, num_devices=8)

    def din(name, shape, dt=F32):
        return nc.dram_tensor(name, shape, dt, kind="ExternalInput").ap()

    xqT = din("xqT", [128, 4, S], BF16)
    xkT = din("xkT", [128, 4, S], BF16)
    xvT = din("xvT", [128, 4, S], BF16)
    wq = din("wq", [128, 4, 256], BF16)
    wk = din("wk", [128, 4, 256], BF16)
    wv = din("wv", [128, 4, 256], BF16)
    wf = din("wf", [128, 2, 512], BF16)   # fused Wh@Wo, local 4 heads
    bq = din("bq", [128, 2])
    bk = din("bk", [128, 2])
    mask01 = din("mask01", [128, NT])   # 1.0 = keep key, 0.0 = padded-out key
    tri01 = din("tri01", [128, 128], BF16)  # 1.0 where k<=q (keep), else 0.0
    ones1 = din("ones1", [1, 64], BF16)

    # partial (4-head) output rows for windows 0..NW-2; host sums the pair
    out = nc.dram_tensor("out", [NW - 1, 4, 128, D], BF16, kind="ExternalOutput").ap()
    # last window is emitted as per-head-pair (2-head) partials so the tail
    # only waits on the hp1 matmuls: [hp, chunk(4), 128, D] (hp1 slot unused)
    out3 = nc.dram_tensor("out3", [2, 4, 128, D], BF16, kind="ExternalOutput").ap()
    # the very last head-pair ships its RAW ctx accumulators (incl. denominator
    # row 64); the host normalizes and projects them, emptying the device tail
    out3r = nc.dram_tensor("out3r", [2, 65, 512], F32, kind="ExternalOutput").ap()
    dbg = os.environ.get("KDBG", "0") == "1"
    if dbg:
        qdbg = nc.dram_tensor("qdbg", [128, 2, S], BF16, kind="ExternalOutput").ap()
        kdbg = nc.dram_tensor("kdbg", [128, 2, S], BF16, kind="ExternalOutput").ap()
        vdbg = nc.dram_tensor("vdbg", [128, NT, HL * 65], BF16, kind="ExternalOutput").ap()
        cdbg = nc.dram_tensor("cdbg", [2, 128, 512], BF16, kind="ExternalOutput").ap()
        adbg = nc.dram_tensor("adbg", [128, 1024], BF16, kind="ExternalOutput").ap()

    from contextlib import ExitStack

    with tile.TileContext(nc) as tc, ExitStack() as ctx:
        # ---- persistent SBUF ----
        pers = ctx.enter_context(tc.tile_pool(name="pers", bufs=1))
        xq_sb = pers.tile([128, 4, S], BF16, tag="xq")
        xk_sb = pers.tile([128, 4, S], BF16, tag="xk")
        xv_sb = pers.tile([128, 4, S], BF16, tag="xv")
        qT_all = pers.tile([128, 2, S], BF16, tag="qT")
        kT_all = pers.tile([128, 2, S], BF16, tag="kT")
        v_sb = pers.tile([128, NT, HL * 65], BF16, tag="vsb")
        wq_sb = pers.tile([128, 4, 256], BF16, tag="wq")
        wk_sb = pers.tile([128, 4, 256], BF16, tag="wk")
        wv_sb = pers.tile([128, 4, 256], BF16, tag="wv")
        wf_sb = pers.tile([128, 2, 512], BF16, tag="wf")
        bq_sb = pers.tile([128, 2], F32, tag="bq")
        bk_sb = pers.tile([128, 2], F32, tag="bk")
        mask_sb = pers.tile([128, NT], F32, tag="mask")
        tri_sb = pers.tile([128, 128], BF16, tag="tri")
        ones1_sb = pers.tile([1, 64], BF16, tag="ones1")

        # ---- pools ----
        atp = ctx.enter_context(tc.tile_pool(name="atp", bufs=10))
        smp = ctx.enter_context(tc.tile_pool(name="smp", bufs=4))
        cxp = ctx.enter_context(tc.tile_pool(name="cxp", bufs=4))
        ostp = ctx.enter_context(tc.tile_pool(name="ostp", bufs=3))
        shr = ctx.enter_context(tc.tile_pool(name="shr", bufs=2, space="PSUM"))
        ppj = ctx.enter_context(tc.tile_pool(name="ppj", bufs=2, space="PSUM"))
        pcx = ctx.enter_context(tc.tile_pool(name="pcx", bufs=2, space="PSUM"))

        # warm up the custom-DVE reciprocal microcode table long before the
        # first real use (first-window normalize raced the lazy table load)
        wrm_i = pers.tile([1, 64], F32, tag="wrm_i")
        wrm_o = pers.tile([1, 64], F32, tag="wrm_o")
        nc.vector.memset(wrm_i, 1.0)
        nc.vector.reciprocal_approx_fast(out=wrm_o, in_=wrm_i)

        # ============ Input DMAs (processing order, w0 first, 2 queues) =====
        # gpsimd carries the weights + odd x-chunks back-to-back; sync carries
        # the tiny biases and even x-chunks, so the w0 q-projection has its
        # operands after ~3 transfers per queue.
        nc.gpsimd.dma_start(out=wq_sb, in_=wq)
        nc.sync.dma_start(out=xq_sb[:, 0, 0:512], in_=xqT[:, 0, 0:512])
        nc.sync.dma_start(out=xq_sb[:, 2, 0:512], in_=xqT[:, 2, 0:512])
        nc.gpsimd.dma_start(out=xq_sb[:, 1, 0:512], in_=xqT[:, 1, 0:512])
        nc.gpsimd.dma_start(out=xq_sb[:, 3, 0:512], in_=xqT[:, 3, 0:512])
        nc.sync.dma_start(out=bq_sb, in_=bq)
        nc.gpsimd.dma_start(out=wk_sb, in_=wk)
        nc.sync.dma_start(out=xk_sb[:, 0, 0:512], in_=xkT[:, 0, 0:512])
        nc.sync.dma_start(out=xk_sb[:, 2, 0:512], in_=xkT[:, 2, 0:512])
        nc.sync.dma_start(out=bk_sb, in_=bk)
        nc.gpsimd.dma_start(out=xk_sb[:, 1, 0:512], in_=xkT[:, 1, 0:512])
        nc.gpsimd.dma_start(out=xk_sb[:, 3, 0:512], in_=xkT[:, 3, 0:512])
        nc.gpsimd.dma_start(out=wv_sb, in_=wv)
        nc.sync.dma_start(out=xv_sb[:, 0, 0:512], in_=xvT[:, 0, 0:512])
        nc.sync.dma_start(out=xv_sb[:, 2, 0:512], in_=xvT[:, 2, 0:512])
        nc.gpsimd.dma_start(out=xv_sb[:, 1, 0:512], in_=xvT[:, 1, 0:512])
        nc.gpsimd.dma_start(out=xv_sb[:, 3, 0:512], in_=xvT[:, 3, 0:512])
        nc.sync.dma_start(out=mask_sb, in_=mask01)
        nc.sync.dma_start(out=tri_sb, in_=tri01)
        nc.sync.dma_start(out=ones1_sb, in_=ones1)
        nc.sync.dma_start(out=wf_sb, in_=wf)
        for w in [1, 2, 3]:
            for si, (src_, dst) in enumerate(((xqT, xq_sb), (xkT, xk_sb),
                                              (xvT, xv_sb))):
                for dc in range(4):
                    eng = nc.sync if (si * 4 + dc) % 2 == 0 else nc.gpsimd
                    eng.dma_start(out=dst[:, dc, w * 512:(w + 1) * 512],
                                  in_=src_[:, dc, w * 512:(w + 1) * 512])

        # ============ Projection units (injected between attention tiles) ===
        def proj_qk_unit(w, xsb, w_sb, b_sb, dst, hc):
            def emit():
                pq = ppj.tile([128, 512], F32, tag="pj")
                for dc in range(4):
                    nc.tensor.matmul(pq, w_sb[:, dc, hc * 128:hc * 128 + 128],
                                     xsb[:, dc, w * 512:(w + 1) * 512],
                                     start=(dc == 0), stop=(dc == 3))
                nc.vector.tensor_scalar_add(
                    out=dst[:, hc, w * 512:(w + 1) * 512], in0=pq,
                    scalar1=b_sb[:, hc:hc + 1])
            return emit

        def proj_v_unit(w, t):
            # bv==0 in this problem, so v = (Xv@Wv) * keep-mask directly from
            # PSUM; col 64 of each head group holds the keep mask (softmax
            # denominator row after the ctx matmul).
            def emit():
                tt = 4 * w + t
                pv = ppj.tile([128, 512], F32, tag="pj")
                for dc in range(4):
                    nc.tensor.matmul(pv[:, 0:256], xv_sb[:, dc, tt * 128:tt * 128 + 128],
                                     wv_sb[:, dc, :], start=(dc == 0), stop=(dc == 3))
                v4 = v_sb[:, tt, :].rearrange("p (h u) -> p h u", u=65)
                nc.vector.tensor_scalar_mul(
                    out=v4[:, :, 0:64],
                    in0=pv[:, 0:256].rearrange("p (h u) -> p h u", u=64),
                    scalar1=mask_sb[:, tt:tt + 1])
                mcol = mask_sb[:, tt:tt + 1]
                mbc = bass.AP(tensor=mcol.tensor, offset=mcol.offset,
                              ap=[mcol.ap[0], [0, HL]])
                nc.vector.tensor_scalar_add(out=v4[:, :, 64], in0=mbc, scalar1=0.0)
            return emit

        def proj_k_chunk_unit(w, hc, chunk):
            """One 128-col slice of the k projection (startup latency hiding)."""
            def emit():
                lo = w * 512 + chunk * 128
                pk = ppj.tile([128, 128], F32, tag="pj", name="pkc")
                for dc in range(4):
                    nc.tensor.matmul(pk, wk_sb[:, dc, hc * 128:hc * 128 + 128],
                                     xk_sb[:, dc, lo:lo + 128],
                                     start=(dc == 0), stop=(dc == 3))
                nc.vector.tensor_scalar_add(
                    out=kT_all[:, hc, lo:lo + 128], in0=pk,
                    scalar1=bk_sb[:, hc:hc + 1])
            return emit

        def proj_units(w, hcs=(0, 1), v=True):
            units = []
            for xsb, w_sb, b_sb, dst in ((xq_sb, wq_sb, bq_sb, qT_all),
                                         (xk_sb, wk_sb, bk_sb, kT_all)):
                for hc in hcs:
                    units.append(proj_qk_unit(w, xsb, w_sb, b_sb, dst, hc))
            if v:
                for t in range(4):
                    units.append(proj_v_unit(w, t))
            return units

        # ============ Normalize (per head-pair): 1/rowsum then scale =========
        def make_normalize(w, hp, pctxA, pctxB, ctxn_slot, use_act=False):
            """Reciprocal of the rowsums (DVE part), then PE broadcast + ctxn
            multiply (PE part). Split so the PE part can be emitted a couple
            of score-tiles later, after the DVE chain has completed.
            use_act: route the copies through the (idle-at-tail) ACT engine,
            per-head pipelined with the DVE reciprocals."""
            hold = []

            def emit_dve():
                if use_act:
                    pieces = []
                    for nmx, pc in (("A", pctxA), ("B", pctxB)):
                        rr = smp.tile([1, 512], F32, tag="rr2", name="rr" + nmx)
                        nc.scalar.copy(out=rr, in_=pc[64:65, :])
                        rc = smp.tile([1, 512], F32, tag="rrc2", name="rc" + nmx)
                        nc.vector.reciprocal_approx_fast(out=rc, in_=rr)
                        rb = smp.tile([1, 512], BF16, tag="rrb2", name="rb" + nmx)
                        nc.scalar.copy(out=rb, in_=rc)
                        pieces.append(rb)
                    hold.append(pieces)
                    return
                rr2 = smp.tile([1, 1024], F32, tag="rr2", name="rr2")
                nc.vector.tensor_scalar_add(out=rr2[:, 0:512], in0=pctxA[64:65, :],
                                            scalar1=0.0)
                nc.vector.tensor_scalar_add(out=rr2[:, 512:1024], in0=pctxB[64:65, :],
                                            scalar1=0.0)
                rrc2 = smp.tile([1, 1024], F32, tag="rrc2", name="rrc2")
                nc.vector.reciprocal_approx_fast(out=rrc2, in_=rr2)
                rrb2 = smp.tile([1, 1024], BF16, tag="rrb2", name="rrb2")
                nc.vector.tensor_scalar_add(out=rrb2, in0=rrc2, scalar1=0.0)
                hold.append(rrb2)

            def emit_pe():
                prb = ppj.tile([128, 512], F32, tag="pj", name="prb")
                if use_act:
                    rbA, rbB = hold[0]
                    nc.tensor.matmul(prb[0:64, :], ones1_sb, rbA,
                                     start=True, stop=True)
                    nc.tensor.matmul(prb[64:128, :], ones1_sb, rbB,
                                     start=True, stop=True)
                else:
                    rrb2 = hold[0]
                    nc.tensor.matmul(prb[0:64, :], ones1_sb, rrb2[:, 0:512],
                                     start=True, stop=True)
                    nc.tensor.matmul(prb[64:128, :], ones1_sb, rrb2[:, 512:1024],
                                     start=True, stop=True)
                rbc = smp.tile([128, 512], BF16, tag="rbc", name="rbc")
                if use_act:
                    nc.scalar.copy(out=rbc, in_=prb)
                else:
                    nc.vector.tensor_scalar_add(out=rbc, in0=prb, scalar1=0.0)
                ctxn2 = cxp.tile([128, 512], BF16, tag="ctxn2", name="ctxn2")
                m1 = nc.vector.tensor_mul(out=ctxn2[0:64, :], in0=pctxA[0:64, :],
                                          in1=rbc[0:64, :])
                m2 = nc.vector.tensor_mul(out=ctxn2[64:128, :], in0=pctxB[0:64, :],
                                          in1=rbc[64:128, :])
                last_ctxn[0] = (m1, m2)
                if dbg and w == 1:
                    nc.sync.dma_start(out=cdbg[hp], in_=ctxn2)
                ctxn_slot[hp] = ctxn2
            return emit_dve, emit_pe

        def out_chunk_unit(w, chunk, ctxn_slot):
            """Partial (4-head) output projection for one 128-q chunk."""
            def emit():
                po = ppj.tile([128, 512], F32, tag="pj", name="po")
                for hp in range(2):
                    nc.tensor.matmul(po, ctxn_slot[hp][:, chunk * 128:chunk * 128 + 128],
                                     wf_sb[:, hp, :], start=(hp == 0), stop=(hp == 1))
                ost = ostp.tile([128, 512], BF16, tag="ost", name="ost")
                nc.vector.tensor_scalar_add(out=ost, in0=po, scalar1=0.0)
                eng = nc.sync if chunk % 2 == 0 else nc.gpsimd
                eng.dma_start(out=out[w, chunk], in_=ost)
            return emit

        def out3_chunk_unit(chunk, hp, ctxn_slot, eng, cp_act=False):
            """Last window: 2-head partial for one chunk of one head-pair."""
            def emit():
                po = ppj.tile([128, 512], F32, tag="pj", name="po3")
                nc.tensor.matmul(po, ctxn_slot[hp][:, chunk * 128:chunk * 128 + 128],
                                 wf_sb[:, hp, :], start=True, stop=True)
                ost = ostp.tile([128, 512], BF16, tag="ost", name="ost3")
                if cp_act:
                    nc.scalar.copy(out=ost, in_=po)
                else:
                    nc.vector.tensor_scalar_add(out=ost, in0=po, scalar1=0.0)
                eng.dma_start(out=out3[hp, chunk], in_=ost)
            return emit

        # ============ Pending-ctx queue (crosses hp/window boundaries) =======
        pend = []   # dicts: at2/qlo/c/w/hp/pA/pB awaiting their ctx matmuls

        def flush_pend(limit):
            while len(pend) > limit:
                e = pend.pop(0)
                at2_, qlo_, c_, w_, hp_ = e["at2"], e["qlo"], e["c"], e["w"], e["hp"]
                j_ = c_ - 4 * w_
                for hi, pctx_ in ((0, e["pA"]), (1, e["pB"])):
                    vsl = v_sb[:, c_, (2 * hp_ + hi) * 65: (2 * hp_ + hi) * 65 + 65]
                    if j_ < 0:
                        mm = nc.tensor.matmul(
                            pctx_[:, qlo_:512], vsl,
                            at2_[:, hi * 512 + qlo_: hi * 512 + 512],
                            start=(c_ == 0), stop=False)
                        if c_ == 0 and last_ctxn[0] is not None:
                            # first write reusing the pctx PSUM slot: wait for
                            # the 2-ago pair's normalize reads (late-emitted,
                            # may miss the pool dep)
                            bass._add_dep_helper(
                                mm.ins, last_ctxn[0][hi].ins, True,
                                "pctx slot reuse vs normalize read")
                    else:
                        # diag tile j_ is the LAST writer of pctx region
                        # [qlo_:qlo_+128] -> give it a proper stop so the
                        # normalize chain's read is synced
                        mm = nc.tensor.matmul(
                            pctx_[:, qlo_:qlo_ + 128], vsl,
                            at2_[:, hi * 512 + qlo_: hi * 512 + qlo_ + 128],
                            start=(c_ == 0), stop=True)
                        if c_ == 0 and last_ctxn[0] is not None:
                            bass._add_dep_helper(
                                mm.ins, last_ctxn[0][hi].ins, True,
                                "pctx slot reuse vs normalize read")
                        if qlo_ + 128 < 512:
                            nc.tensor.matmul(
                                pctx_[:, qlo_ + 128:512], vsl,
                                at2_[:, hi * 512 + qlo_ + 128: hi * 512 + 512],
                                start=(c_ == 0), stop=False)

        def flush_old(w, hp):
            """Flush every entry not belonging to head-pair (w, hp)."""
            while pend and not (pend[0]["w"] == w and pend[0]["hp"] == hp):
                flush_pend(len(pend) - 1)

        # ============ Attention windows ======================================
        def emit_attention(w, inject, ctxn_slot, post_hp0=None, pre_tile=None,
                           last_w=False):
            """inject: closures spread between tiles; scores run a few tiles
            ahead of ctx; boundary-tagged inject[0..1] run at (hp0, tiles 1/3).
            post_hp0: units queued right behind hp0's deferred normalize (they
            read hp0's ctxn, so they may only drain once nz_pe has run).
            pre_tile: {(hp, c): [units]} emitted before that tile's scores
            (units the tile's own scores depend on - must precede in-stream).
            The trailing ctx flushes of each head-pair are carried ACROSS the
            hp/window boundary (global pend queue) so the next pair's scores
            and exps stream while the old pair's ctx burst + normalize run."""
            n = 4 * (w + 1)

            def drain(k):
                # boundary-tagged closures only run at their tile slot
                while k > 0 and inject and not getattr(inject[0], "boundary", 0):
                    inject.pop(0)()
                    k -= 1

            for hp in range(2):
                # per-hp drain budget: recompute after hp0's inserts so units
                # added mid-window (normalize, out3 partials) drain promptly
                spread = len(inject)
                tix = 0
                emitted = [0]
                pctxA = pcx.tile([65, 512], F32, tag="ctx", name="pctxA")
                pctxB = pcx.tile([65, 512], F32, tag="ctx", name="pctxB")

                for c in range(n):
                    if pre_tile:
                        for u_ in pre_tile.pop((hp, c), []):
                            u_()
                    if c == 1:
                        # the old pair's remaining ctx must land before its
                        # normalize (boundary-popped right below)
                        flush_old(w, hp)
                    j = c - 4 * w
                    qlo = max(0, 128 * j)
                    ps2 = shr.tile([128, 1024], F32, tag="big", name="ps2")
                    at2 = atp.tile([128, 1024], BF16, tag="at", name="at2")
                    for hi in range(2):
                        kT_c = kT_all[64 * hi: 64 * hi + 64, hp, c * 128: c * 128 + 128]
                        nc.tensor.matmul(
                            ps2[:, hi * 512 + qlo: hi * 512 + 512], kT_c,
                            qT_all[64 * hi: 64 * hi + 64, hp,
                                   w * 512 + qlo: (w + 1) * 512],
                            start=True, stop=True)
                    if j >= 0:
                        ps3 = ps2.rearrange("p (h q) -> p h q", q=512)
                        at3 = at2.rearrange("p (h q) -> p h q", q=512)
                        nc.scalar.activation(out=at3[:, :, qlo:512],
                                             in_=ps3[:, :, qlo:512],
                                             func=EXP, bias=0.0, scale=0.125)
                        # causal mask on the 128-wide diagonal block of both
                        # heads: one strided in-place DVE multiply by tri01
                        dg = at3[:, :, qlo:qlo + 128]
                        trib = bass.AP(tensor=tri_sb.tensor, offset=tri_sb.offset,
                                       ap=[tri_sb.ap[0], [0, 2], tri_sb.ap[1]])
                        nc.vector.tensor_mul(out=dg, in0=dg, in1=trib)
                    else:
                        nc.scalar.activation(out=at2, in_=ps2,
                                             func=EXP, bias=0.0, scale=0.125)
                    if dbg and w == 1 and hp == 0 and c == 4:
                        nc.sync.dma_start(out=adbg, in_=at2)
                    pend.append(dict(at2=at2, qlo=qlo, c=c, w=w, hp=hp,
                                     pA=pctxA, pB=pctxB))
                    while (c in (1, 3) and inject
                           and getattr(inject[0], "boundary", 0) == (1 if c == 1 else 2)):
                        inject.pop(0)()   # deferred normalize of previous pair
                    flush_pend(7)
                    tix += 1
                    # hp0 drains ~half the queue; hp1 drains the rest (incl.
                    # units inserted at the hp boundary)
                    want = (tix * spread) // (n * (2 - hp))
                    if want > emitted[0]:
                        drain(want - emitted[0])
                        emitted[0] = want
                if last_w and hp == 1:
                    # no on-device normalize for the final pair: ship raw
                    last_pctx[0] = (pctxA, pctxB)
                    continue
                nz_dve, nz_pe = make_normalize(w, hp, pctxA, pctxB, ctxn_slot)
                nz_dve.boundary = 1
                nz_pe.boundary = 2
                if hp == 0:
                    # defer hp0's normalize into hp1's score stream
                    inject.insert(0, nz_pe)
                    inject.insert(0, nz_dve)
                    if post_hp0:
                        for k_, u_ in enumerate(post_hp0):
                            inject.insert(2 + k_, u_)
                else:
                    last_nz[0] = (nz_dve, nz_pe)
            # leftovers are carried into the next window's schedule so they
            # don't pile up in front of its first scores
            return inject

        last_nz = [None]
        last_ctxn = [None]
        last_pctx = [None]
        # only the hp0-critical projections run before attention; the rest of
        # w0's projections are injected so scores start as soon as DMAs land.
        # k-hc0 is emitted per 128-col chunk right before the tile needing it.
        for u in proj_units(0, hcs=(0,), v=False)[:1]:
            u()                               # q-hc0 (full window)
        proj_k_chunk_unit(0, 0, 0)()          # k-hc0 chunk 0
        ctxn_slots = [[None, None] for _ in range(NW)]
        carry = []
        for w in range(NW):
            inject = []
            if last_nz[0] is not None:
                inject.extend(last_nz[0])   # normalize of previous window's hp1
                last_nz[0] = None
            inject += carry
            rest = []
            if w == 0:
                rest += proj_units(0, hcs=(1,), v=False)
                rest += proj_units(0, hcs=(), v=True)
            if w >= 1:
                rest += [out_chunk_unit(w - 1, ch, ctxn_slots[w - 1])
                         for ch in range(4)]
            if w + 1 < NW:
                rest += proj_units(w + 1)
            # interleave: proj, proj, out, proj, proj, out, ... so no two
            # out-chunks (which share the ppj pool) are adjacent
            if w >= 1 and w + 1 < NW:
                og = rest[:4]
                pg = rest[4:]
                mixed = []
                pi = oi = 0
                for k in range(len(rest)):
                    if k % 3 == 2 and oi < len(og):
                        mixed.append(og[oi]); oi += 1
                    elif pi < len(pg):
                        mixed.append(pg[pi]); pi += 1
                    elif oi < len(og):
                        mixed.append(og[oi]); oi += 1
                rest = mixed
            inject += rest
            post_hp0 = None
            pre_tile = None
            if w == 0:
                pre_tile = {(0, c): [proj_k_chunk_unit(0, 0, c)]
                            for c in (1, 2, 3)}
            if w == NW - 1:
                engs3 = (nc.sync, nc.gpsimd, nc.sync, nc.gpsimd)
                post_hp0 = [out3_chunk_unit(ch, 0, ctxn_slots[w], engs3[ch])
                            for ch in range(4)]
            carry = emit_attention(w, inject, ctxn_slots[w], post_hp0, pre_tile,
                                   last_w=(w == NW - 1))
        for u in carry:
            u()
        flush_pend(0)   # last head-pair's trailing ctx
        # device tail = stage the raw ctx accumulators to SBUF (ACT and DVE in
        # parallel) and store; host normalizes + projects them
        rawA = pers.tile([65, 512], F32, tag="rawA")
        rawB = pers.tile([65, 512], F32, tag="rawB")
        nc.scalar.copy(out=rawA, in_=last_pctx[0][0])
        nc.vector.tensor_scalar_add(out=rawB, in0=last_pctx[0][1], scalar1=0.0)
        nc.sync.dma_start(out=out3r[0], in_=rawA)
        nc.gpsimd.dma_start(out=out3r[1], in_=rawB)
        if dbg:
            nc.sync.dma_start(out=qdbg, in_=qT_all)
            nc.sync.dma_start(out=kdbg, in_=kT_all)
            nc.sync.dma_start(out=vdbg, in_=v_sb)

    nc.compile()
    return nc


_NC = None


def _get_nc():
    global _NC
    if _NC is None:
        _NC = build_program()
    return _NC


def make_core_inputs(Q, K, V, padding_mask, Wq, bq, Wk, bk, Wv, bv, Wh, bh, Wo, bo):
    """Shard the full problem inputs into 8 per-core input dicts."""
    f = np.float32
    bf = mybir.dt.np(BF16)
    # keep mask for the diag block: rows = k_local, cols = q_local, keep k<=q
    tri = np.triu(np.ones((128, 128), f), 0).astype(bf)
    ones1 = np.ones((1, 64), f)
    Wo = np.asarray(Wo, f)
    Wh_ = np.asarray(Wh, f)

    def chunk_xT(x):  # [S, D] -> [128, 4, S]
        return np.ascontiguousarray(
            np.asarray(x, f).T.reshape(4, 128, S).transpose(1, 0, 2)).astype(bf)

    ins = []
    for c in range(8):
        b, quad = c // 2, c % 2
        hlo = quad * HL
        wq_c = np.ascontiguousarray(np.transpose(np.asarray(Wq, f)[hlo:hlo + HL], (1, 0, 2))
                                    ).reshape(D, HL * DK)
        wk_c = np.ascontiguousarray(np.transpose(np.asarray(Wk, f)[hlo:hlo + HL], (1, 0, 2))
                                    ).reshape(D, HL * DK)
        wv_c = np.ascontiguousarray(np.transpose(np.asarray(Wv, f)[hlo:hlo + HL], (1, 0, 2))
                                    ).reshape(D, HL * DV)
        bq_c = np.asarray(bq, f)[hlo:hlo + HL].reshape(-1)
        bk_c = np.asarray(bk, f)[hlo:hlo + HL].reshape(-1)
        # fused (Wh_h @ Wo_rows_h) for the LOCAL 4 heads: [128, 2, 512]
        wf_in = np.zeros((128, 2, D), f)
        for lh in range(HL):
            h = hlo + lh
            hpp, l = lh // 2, lh % 2
            wf_in[64 * l: 64 * l + 64, hpp, :] = \
                Wh_[h] @ Wo[h * DV:(h + 1) * DV, :]
        pm = np.asarray(padding_mask[b, 0])
        keep = np.where(pm, np.float32(0.0), np.float32(1.0)).astype(f)
        ins.append({
            "xqT": chunk_xT(np.asarray(Q, f)[b]),
            "xkT": chunk_xT(np.asarray(K, f)[b]),
            "xvT": chunk_xT(np.asarray(V, f)[b]),
            "wq": np.ascontiguousarray(wq_c.reshape(4, 128, 256).transpose(1, 0, 2)).astype(bf),
            "wk": np.ascontiguousarray(wk_c.reshape(4, 128, 256).transpose(1, 0, 2)).astype(bf),
            "wv": np.ascontiguousarray(wv_c.reshape(4, 128, 256).transpose(1, 0, 2)).astype(bf),
            "wf": wf_in.astype(bf),
            "bq": np.ascontiguousarray(bq_c.reshape(2, 128).T),
            "bk": np.ascontiguousarray(bk_c.reshape(2, 128).T),
            "mask01": np.ascontiguousarray(keep.reshape(NT, 128).T),
            "tri01": tri,
            "ones1": ones1.astype(bf),
        })
    return ins


def run(inputs_list, **kw):
    nc = _get_nc()
    return bass_utils.run_bass_kernel_spmd(nc, inputs_list, core_ids=list(range(8)), **kw)


def kernel(Q, K, V, padding_mask, Wq, bq, Wk, bk, Wv, bv, Wh, bh, Wo, bo):
    ins = make_core_inputs(Q, K, V, padding_mask, Wq, bq, Wk, bk, Wv, bv, Wh, bh, Wo, bo)
    res = run(ins)
    f = np.float32
    # fused output bias: bo + sum_h bh_h @ Wo_h
    Wo_ = np.asarray(Wo, f)
    bias = np.asarray(bo, f).copy()
    for h in range(H):
        bias = bias + np.asarray(bh, f)[h] @ Wo_[h * DV:(h + 1) * DV, :]
    Wh_ = np.asarray(Wh, f)
    out = np.empty((B, S, D), f)
    lo3 = (NW - 1) * 512
    for b in range(B):
        ra, rb = res.results[2 * b], res.results[2 * b + 1]
        pa = np.asarray(ra["out"], dtype=f)        # [NW-1,4,128,D]
        pb = np.asarray(rb["out"], dtype=f)
        out[b, :lo3] = (pa + pb).reshape(lo3, D) + bias
        # last window: device hp0 partials + host-normalized raw hp1 pair
        p3 = (np.asarray(ra["out3"], dtype=f)[0]
              + np.asarray(rb["out3"], dtype=f)[0]).reshape(512, D)
        for quad, rr in ((0, ra), (1, rb)):
            raw = np.asarray(rr["out3r"], dtype=f)     # [2, 65, 512]
            for hi in range(2):
                h = quad * HL + 2 + hi
                ctxn = raw[hi, 0:64] / raw[hi, 64:65]   # [64 dv, 512 q]
                p3 = p3 + ctxn.T @ (Wh_[h] @ Wo_[h * DV:(h + 1) * DV, :])
        out[b, lo3:] = p3 + bias
    return out
